# revision 100
# baseline (speedup 1.0000x reference)
"""Distributed Trainium2 Bass kernel for nn_Attention_57346403336225.

Reference computation (per batch b of 16, N=1024 tokens, E=128 emb, H=8 heads,
head dim d = E = 128, INNER = 1024):
    xn   = LayerNorm(x) * ln_w + ln_b
    qkv  = xn @ w_qkv ; q,k,v heads
    dots = (q @ k^T) * scale[h]  ; diagonal masked to -FLT_MAX
    attn = softmax(dots) ; out = attn @ v
    y    = out @ w_proj + b_proj

Sharding: pure data-parallel over batch (16 / 8 cores = 2 batches per core),
weights replicated, no collectives.

Hybrid head scheduling (v2): the kernel alternates two per-(batch,head)
pipelines to balance engine load:

  R-heads (row-major): dots = (A_h @ xn^T)^T-style row-major tiles; exp on
    ScalarE with fused row-sum accumulation; P^T obtained via DMA xbar
    transposes (descriptor-heavy: ~4.2k descriptors per [128,4096] call,
    saturating all 16 DMA engines ~5us per call).

  T-heads (transposed): dots^T computed directly via uT = A_h^T @ xn^T, so
    exp output IS P^T and no DMA transpose is needed. Row sums (softmax
    denominators) are recovered by a DVE pairwise tree over the 8 key tiles
    (bf16 partials - numerically validated exact at rel 3.6e-3) followed by
    8 single-column partition-reduce matmuls (st-slice^T @ ones) into a
    [128,8] PSUM tile and a DVE reciprocal.  (A GpSimd partition_all_reduce
    measured 6.7us/call and head-of-line-blocked the PE queue via the
    downstream PE transpose - matmul reduction avoids both.)

  The T/R mix trades DMA-descriptor throughput (the baseline wall) against
  DVE tree work; ScalarE exp (~8.3us/iter) is common to both.

Other changes vs v1: diagonal mask applied by a tiny accumulating matmul
(-1e30*I) on the PE instead of DVE adds; both batches' LN stats on DVE
bn_stats (ScalarE keeps only Sqrt in the prologue + the 128 Exp tiles).
"""

import numpy as np
import ml_dtypes

B, N, E, H = 16, 1024, 128, 8
NCORES = 8
B_LOC = B // NCORES  # 2
LN_EPS = 1e-5
NT = N // 128    # 8 token tiles per batch
MASK_VAL = -1e30

_cache = {}

# iteration -> transposed-head? (alternating, f=0.5)
NIT_TOTAL = B_LOC * H


def _is_t(it):
    return it % 2 == 0


def _build_nc():
    import concourse.bacc as bacc
    import concourse.mybir as mybir
    import concourse.tile as tile

    f32 = mybir.dt.float32
    bf16 = mybir.dt.bfloat16
    Exp = mybir.ActivationFunctionType.Exp
    Sqrt = mybir.ActivationFunctionType.Sqrt
    Ident = mybir.ActivationFunctionType.Identity
    sub = mybir.AluOpType.subtract
    mult = mybir.AluOpType.mult
    add = mybir.AluOpType.add

    nc = bacc.Bacc("TRN2", target_bir_lowering=False)

    x_p = nc.declare_dram_parameter("x", [B_LOC, N, E], f32, isOutput=False)
    a_p = nc.declare_dram_parameter("amat", [H, E, E], bf16, isOutput=False)
    at_p = nc.declare_dram_parameter("amatT", [H, E, E], bf16, isOutput=False)
    wvf_p = nc.declare_dram_parameter("wvf", [E, H * E], bf16, isOutput=False)
    wp_p = nc.declare_dram_parameter("wp", [H, E, E], bf16, isOutput=False)
    lnw_p = nc.declare_dram_parameter("lnw", [E, 1], f32, isOutput=False)
    lnb_p = nc.declare_dram_parameter("lnb", [E, 1], f32, isOutput=False)
    id_p = nc.declare_dram_parameter("ident", [E, E], f32, isOutput=False)
    idb_p = nc.declare_dram_parameter("identb", [E, E], bf16, isOutput=False)
    nib_p = nc.declare_dram_parameter("negib", [E, E], bf16, isOutput=False)
    bp_p = nc.declare_dram_parameter("bptile", [128, E], f32, isOutput=False)
    out_p = nc.declare_dram_parameter("out", [B_LOC, N, E], f32, isOutput=True)

    with tile.TileContext(nc) as tc:
        with (
            tc.tile_pool(name="const", bufs=1) as cpool,
            tc.tile_pool(name="ln", bufs=8) as lnpool,
            tc.tile_pool(name="work", bufs=4) as wpool,
            tc.tile_pool(name="bigP", bufs=2) as ppool,
            tc.tile_pool(name="bigPT", bufs=4) as ptpool,
            tc.tile_pool(name="tpt", bufs=2) as tptpool,
            tc.tile_pool(name="tree", bufs=1) as treepool,
            tc.tile_pool(name="psd", bufs=2, space="PSUM") as psd,
            tc.tile_pool(name="psm", bufs=2, space="PSUM") as psm,
        ):
            # ---- constants ----
            ident = cpool.tile([E, E], f32, tag="ident")
            identb = cpool.tile([E, E], bf16, tag="identb")
            negib = cpool.tile([E, E], bf16, tag="negib")
            lnw = cpool.tile([E, 1], f32, tag="lnw")
            lnb = cpool.tile([E, 1], f32, tag="lnb")
            amat = cpool.tile([E, H, E], bf16, tag="amat")
            amatT = cpool.tile([E, H, E], bf16, tag="amatT")
            wvf = cpool.tile([E, H * E], bf16, tag="wvf")
            wp = cpool.tile([E, H, E], bf16, tag="wp")
            bptile = cpool.tile([128, E], f32, tag="bptile")
            epst = cpool.tile([128, 1], f32, tag="epst")
            zbias = cpool.tile([128, 1], f32, tag="zbias")
            onesb = cpool.tile([128, 1], bf16, tag="onesb")
            xall = [cpool.tile([128, NT, E], f32, tag=f"xall{b}",
                               name=f"xall{b}") for b in range(B_LOC)]
            xnT = [cpool.tile([E, N], bf16, tag=f"xnT{b}", name=f"xnT{b}")
                   for b in range(B_LOC)]
            vall = [cpool.tile([128, NT, H * E], bf16, tag=f"vall{b}",
                               name=f"vall{b}") for b in range(B_LOC)]
            y_acc = [cpool.tile([128, N], f32, tag=f"yacc{b}", name=f"yacc{b}")
                     for b in range(B_LOC)]
            # per-token LN coefficients: xn = x * rs - mur
            rs8 = [cpool.tile([128, NT], f32, tag=f"rs8{b}", name=f"rs8{b}")
                   for b in range(B_LOC)]
            mur8 = [cpool.tile([128, NT], f32, tag=f"mur8{b}",
                               name=f"mur8{b}") for b in range(B_LOC)]

            # batch-0 x first, split across BOTH DMA queues so the two
            # halves stream in parallel and LayerNorm stats start sooner
            nc.sync.dma_start(
                xall[0][:, 0:4, :],
                x_p[0, 0:512].rearrange("(t p) e -> p t e", p=128))
            nc.gpsimd.dma_start(
                xall[0][:, 4:NT, :],
                x_p[0, 512:N].rearrange("(t p) e -> p t e", p=128))
            nc.sync.dma_start(ident[:], id_p[:])
            nc.sync.dma_start(identb[:], idb_p[:])
            nc.sync.dma_start(negib[:], nib_p[:])
            nc.sync.dma_start(lnw[:], lnw_p[:])
            nc.sync.dma_start(lnb[:], lnb_p[:])
            # amatT before everything else on gpsimd: iteration 0 is a
            # T-head, make_lhs(0) needs it right after xnT[0] is ready
            nc.gpsimd.dma_start(amatT[:], at_p[:].rearrange("h a b -> a h b"))
            nc.gpsimd.dma_start(wvf[:], wvf_p[:])
            nc.gpsimd.dma_start(
                xall[1][:], x_p[1].rearrange("(t p) e -> p t e", p=128))
            nc.gpsimd.dma_start(amat[:], a_p[:].rearrange("h a b -> a h b"))
            nc.gpsimd.dma_start(wp[:], wp_p[:].rearrange("h a b -> a h b"))
            nc.gpsimd.dma_start(bptile[:], bp_p[:])
            nc.vector.memset(epst[:], LN_EPS)
            nc.vector.memset(zbias[:], 0.0)
            nc.vector.memset(onesb[:], 1.0)

            def cast_psum(eng, out, in_):
                """PSUM f32 -> SBUF bf16 cast."""
                if eng is nc.scalar:
                    eng.copy(out, in_)
                else:
                    eng.tensor_copy(out, in_)

            # ---- LN stats on DVE, batched Sqrt on ScalarE (batch 0 first;
            # batch-1 stats wait on the batch-1 x DMA, so emitting them
            # before ln_apply(0,*) would stall the in-order DVE queue)
            mvall = [cpool.tile([128, NT, 2], f32, tag=f"mv{b}",
                                name=f"mv{b}") for b in range(B_LOC)]
            sd = [cpool.tile([128, NT], f32, tag=f"sd{b}", name=f"sd{b}")
                  for b in range(B_LOC)]
            nmur8 = cpool.tile([128, NT], f32, tag="nmur8")

            def ln_stats(b, lo=0, hi=NT):
                stl = {}
                for t in range(lo, hi):
                    st = lnpool.tile([128, 6], f32, tag="st",
                                     name=f"st{b}_{t}")
                    nc.vector.bn_stats(st[:], xall[b][:, t, :])
                    stl[t] = st
                for t in range(lo, hi):
                    nc.vector.bn_aggr(mvall[b][:, t, :], stl[t][:])
                nc.scalar.activation(sd[b][:, lo:hi], mvall[b][:, lo:hi, 1],
                                     Sqrt, bias=epst[:])
                nc.vector.reciprocal(rs8[b][:, lo:hi], sd[b][:, lo:hi])
                nc.vector.tensor_tensor(mur8[b][:, lo:hi],
                                        mvall[b][:, lo:hi, 0],
                                        rs8[b][:, lo:hi], op=mult)
                if b == 0:
                    # negated for the ScalarE-Identity prologue ln_apply
                    nc.vector.tensor_scalar(
                        nmur8[:, lo:hi], mur8[b][:, lo:hi], -1.0, None,
                        op0=mult)

            ln_stats(0, 0, 4)

            def ln_apply(b, t, on_scalar=False):
                """xn = x*rs - mur, PE transpose, ln_w/ln_b epilogue.  The
                prologue (batch 0) runs both affines on the otherwise-idle
                ScalarE (activation Identity with per-partition scale/bias
                APs), keeping the serial LN chain off the DVE queue and
                pulling the first exp ~6us earlier."""
                xn = lnpool.tile([128, E], f32, tag="xnt", name=f"xn{b}_{t}")
                if on_scalar:
                    nc.scalar.activation(
                        xn[:], xall[b][:, t, :], Ident,
                        bias=nmur8[:, t:t + 1], scale=rs8[b][:, t:t + 1],
                    )
                else:
                    nc.vector.tensor_scalar(
                        xn[:], xall[b][:, t, :], rs8[b][:, t:t + 1],
                        mur8[b][:, t:t + 1], op0=mult, op1=sub,
                    )
                tp = psm.tile([128, E], f32, tag="m512", name=f"lntp{b}_{t}")
                nc.tensor.transpose(tp[:], xn[:], ident[:])
                if on_scalar:
                    nc.scalar.activation(
                        xnT[b][:, t * 128:(t + 1) * 128], tp[:], Ident,
                        bias=lnb[:], scale=lnw[:],
                    )
                else:
                    nc.vector.tensor_scalar(
                        xnT[b][:, t * 128:(t + 1) * 128], tp[:],
                        lnw[:], lnb[:], op0=mult, op1=add,
                    )

            def v_proj(b, ts_list, cast_eng=None):
                """v = xn @ Wv for token tiles (all heads).  Batch-0 casts go
                to ScalarE (idle during the prologue; on DVE they'd delay
                iteration 1's tT cast by ~10us), batch-1 casts to DVE."""
                if cast_eng is None:
                    cast_eng = nc.vector
                for t in ts_list:
                    for c in range(2):
                        vps = psm.tile([128, 512], f32, tag="m512",
                                       name=f"vps{b}_{t}_{c}")
                        nc.tensor.matmul(
                            vps[:],
                            xnT[b][:, t * 128:(t + 1) * 128],
                            wvf[:, c * 512:(c + 1) * 512],
                            start=True, stop=True,
                        )
                        cast_psum(cast_eng,
                                  vall[b][:, t, c * 512:(c + 1) * 512],
                                  vps[:])

            # alternate LN tiles between ScalarE and DVE: the all-ScalarE
            # chain ran 6.8us serially while the DVE idled after stats;
            # the two independent chains halve the prologue critical path
            for t in range(4):
                ln_apply(0, t, on_scalar=(t % 2 == 0))
            ln_stats(0, 4, NT)
            for t in range(4, NT):
                ln_apply(0, t, on_scalar=(t % 2 == 0))

            # ---- attention, software-pipelined across (batch, head) ----
            iters = [(b, h) for b in range(B_LOC) for h in range(H)]
            NIT = len(iters)
            stash = {}

            def make_lhs(it):
                """tT = A_h @ xnT (R-heads) or uT = A_h^T @ xnT (T-heads)."""
                b, h = iters[it]
                src = amatT if _is_t(it) else amat
                tT = wpool.tile([E, N], bf16, tag="tT", name=f"tT{it}")
                for qc in range(2):
                    tps = psm.tile([128, 512], f32, tag="m512",
                                   name=f"tps{it}_{qc}")
                    nc.tensor.matmul(
                        tps[:], src[:, h, :],
                        xnT[b][:, qc * 512:(qc + 1) * 512],
                        start=True, stop=True,
                    )
                    cast_psum(nc.vector, tT[:, qc * 512:(qc + 1) * 512],
                              tps[:])
                stash[("tT", it)] = tT

            def mask_mm(dps, blk):
                """dps[:, blk*128:(blk+1)*128] += -1e30 * I via PE."""
                nc.tensor.matmul(
                    dps[:, blk * 128:(blk + 1) * 128],
                    negib[:], identb[:],
                    start=False, stop=True, skip_group_check=True,
                )

            # ---------------- PV machinery (shared) ---------------------
            def pv_start(it, qc):
                ops = psm.tile([128, 512], f32, tag="ops", bufs=2,
                               name=f"ops{it}_{qc}")
                stash[("ops", it, qc)] = ops

            def pv_mm(it, qc, kt):
                b, h = iters[it]
                if _is_t(it):
                    rhs = stash[("PTt", it)][:, kt, qc * 512:(qc + 1) * 512]
                else:
                    rhs = stash[("PT", it, qc)][:, 4 * kt:4 * (kt + 1), :]
                nc.tensor.matmul(
                    stash[("ops", it, qc)][:],
                    vall[b][:, kt, h * E:(h + 1) * E],
                    rhs,
                    start=(kt == 0), stop=(kt == NT - 1),
                )

            def pv_finish(it, qc, eng=None):
                oT = stash[("oT", it)]
                ops = stash.pop(("ops", it, qc))
                cast_psum(eng or nc.vector,
                          oT[:, qc * 512:(qc + 1) * 512], ops[:])

            def pv_plain(it, qc):
                pv_start(it, qc)
                for kt in range(NT):
                    pv_mm(it, qc, kt)
                pv_finish(it, qc)

            # ------------- dots+exp tile (shared R/T) -------------------
            def dots_tile(it, j, pv_slots):
                """One dots tile j (query tile for R-heads, key tile for
                T-heads): 2 matmuls + diag-mask mm + exp.  pv_slots is a
                list of kt indices whose PV matmuls (chunk j//4 of the
                previous iteration) are woven between the non-accumulating
                dots mms (dodges the same-bank PSUM RMW bubble)."""
                b, h = iters[it]
                lhs = stash[("tT", it)]
                pv_it, g = stash.get(("pvctx", it), (None, None))
                dps = psd.tile([128, N], f32, tag="dots",
                               name=f"dps{it}_{j}")
                for kc in range(2):
                    nc.tensor.matmul(
                        dps[:, kc * 512:(kc + 1) * 512],
                        lhs[:, j * 128:(j + 1) * 128],
                        xnT[b][:, kc * 512:(kc + 1) * 512],
                        start=True, stop=False, skip_group_check=True,
                    )
                    if kc < len(pv_slots):
                        pv_mm(pv_it, g, pv_slots[kc])
                mask_mm(dps, j)
                for kt in pv_slots[2:]:
                    pv_mm(pv_it, g, kt)
                if _is_t(it):
                    nc.scalar.activation(
                        stash[("PTt", it)][:, j, :], dps[:], Exp,
                        bias=zbias[:],
                    )
                else:
                    P, rsum = stash[("Pr", it)]
                    Pg = P[:, j // 4, :].rearrange(
                        "p (kt a c) -> p kt a c", a=4, c=128)
                    nc.scalar.activation(
                        Pg[:, :, j % 4, :],
                        dps[:].rearrange("p (kt c) -> p kt c", c=128),
                        Exp, bias=zbias[:],
                        accum_out=rsum[:, j:j + 1],
                    )

            def dots_half(it, half, pv_it, skip_n=0, mid_cb=None):
                """Tiles [4*half, 4*half+4) of iteration it with the PV
                chunk-half matmuls of pv_it interleaved.  skip_n: leading
                tiles already emitted at the end of the previous iteration
                (boundary pipelining).  mid_cb: emitted before the last
                tile (used to pre-emit the next iteration's first dots tile
                early enough in the PE stream that its exp is ready the
                moment ScalarE finishes this iteration's exps)."""
                tiles = list(range(4 * half, 4 * half + 4))[skip_n:]
                if pv_it is not None:
                    pv_start(pv_it, half)
                    stash[("pvctx", it)] = (pv_it, half)
                    # distribute the 8 PV matmuls over the tiles
                    n, rem = divmod(NT, len(tiles))
                    kts = iter(range(NT))
                    slots = [[next(kts) for _ in range(n + (i < rem))]
                             for i in range(len(tiles))]
                else:
                    stash[("pvctx", it)] = (None, None)
                    slots = [[] for _ in tiles]
                for i, (tile_j, sl) in enumerate(zip(tiles, slots)):
                    if mid_cb is not None and i == len(tiles) - 1:
                        mid_cb()
                    dots_tile(it, tile_j, sl)
                if pv_it is not None:
                    # NOTE: routing this cast to ScalarE measured +12us
                    # Scalar busy for 8 casts - mid-loop non-Exp activations
                    # thrash the ACT table set (~2x 1.3us reloads per swap).
                    # ScalarE is effectively Exp-only during the steady
                    # state; all casts stay on DVE.
                    pv_finish(pv_it, half)
                stash.pop(("pvctx", it))
                if not _is_t(it):
                    P, _ = stash[("Pr", it)]
                    PTc = ptpool.tile([128, 4 * NT, 128], bf16, tag="PT",
                                      name=f"PTc{it}_{half}")
                    nc.sync.dma_start(
                        out=PTc[:],
                        in_=P[:, half, :],
                        transpose=True,
                    )
                    stash[("PT", it, half)] = PTc

            def tree_l1(it):
                """Rowsum tree level 1 (2.1us DVE), emitted in the FOLLOWING
                (R) iteration's window on iteration-old exps."""
                PTt = stash[("PTt", it)]
                l1 = treepool.tile([128, 4, N], bf16, tag="l1", name=f"l1{it}")
                pv4 = PTt[:].rearrange("p (f two) n -> p f two n", two=2)
                nc.vector.tensor_tensor(l1[:], pv4[:, :, 0, :],
                                        pv4[:, :, 1, :], op=add)
                stash[("l1", it)] = l1

            def tree_l23(it):
                """Tree levels 2+3 (1.6us DVE), emitted two iterations after
                `it` - in a T window, which has DVE slack.  Splitting the
                tree this way makes both window types ScalarE-paced (the
                whole tree in one R window made it DVE-paced at 10.1us)."""
                l1 = stash.pop(("l1", it))
                l2 = treepool.tile([128, 2, N], bf16, tag="l2", name=f"l2{it}")
                st = treepool.tile([128, N], bf16, tag="st", name=f"sT{it}")
                l14 = l1[:].rearrange("p (f two) n -> p f two n", two=2)
                nc.vector.tensor_tensor(l2[:], l14[:, :, 0, :],
                                        l14[:, :, 1, :], op=add)
                nc.vector.tensor_tensor(st[:], l2[:, 0, :], l2[:, 1, :],
                                        op=add)
                stash[("st", it)] = st

            def finish_rowsum_t(it):
                """8 single-column partition-reduce matmuls (st-slice^T @
                ones) into [128,8] PSUM, then DVE reciprocal."""
                st = stash.pop(("st", it))
                rsT = psm.tile([128, 8], f32, tag="m512", name=f"rsT{it}")
                for j in range(NT):
                    nc.tensor.matmul(
                        rsT[:, j:j + 1],
                        st[:, j * 128:(j + 1) * 128],
                        onesb[:],
                        start=True, stop=True,
                    )
                rcp = wpool.tile([128, NT], f32, tag="rcp", name=f"rcp{it}")
                nc.vector.reciprocal(rcp[:], rsT[:])
                stash[("rcp", it)] = rcp

            def proj_mm(it, t):
                b, h = iters[it]
                oT = stash[("oT", it)]
                yps = psm.tile([128, E], f32, tag="m512",
                               name=f"yps{it}_{t}")
                nc.tensor.matmul(
                    yps[:],
                    oT[:, t * 128:(t + 1) * 128],
                    wp[:, h, :],
                    start=True, stop=True,
                )
                stash[("yps", it, t)] = yps

            def proj_ep(it, t):
                b, h = iters[it]
                rcp = stash[("rcp", it)]
                yps = stash.pop(("yps", it, t))
                if h == 0:
                    nc.vector.scalar_tensor_tensor(
                        y_acc[b][:, t * 128:(t + 1) * 128],
                        yps[:], rcp[:, t:t + 1], bptile[:],
                        op0=mult, op1=add,
                    )
                else:
                    nc.vector.scalar_tensor_tensor(
                        y_acc[b][:, t * 128:(t + 1) * 128],
                        yps[:], rcp[:, t:t + 1],
                        y_acc[b][:, t * 128:(t + 1) * 128],
                        op0=mult, op1=add,
                    )
                if h == H - 1 and t % 2 == 1:
                    nc.gpsimd.dma_start(
                        out_p[b, (t - 1) * 128:(t + 1) * 128].rearrange(
                            "(u p) e -> p u e", p=128),
                        y_acc[b][:, (t - 1) * 128:(t + 1) * 128].rearrange(
                            "p (u e) -> p u e", u=2),
                    )

            def proj_drop(it):
                stash.pop(("oT", it))
                stash.pop(("rcp", it))
                stash.pop(("tT", it))
                if _is_t(it):
                    stash.pop(("PTt", it))
                else:
                    stash.pop(("PT", it, 0))
                    stash.pop(("PT", it, 1))

            def emit_rowsum(it):
                """Denominator reciprocals for iteration it (consumed by the
                projection epilogue two iterations later)."""
                if _is_t(it):
                    finish_rowsum_t(it)
                else:
                    _, rsum = stash.pop(("Pr", it))
                    rcp = wpool.tile([128, NT], f32, tag="rcp",
                                     name=f"rcp{it}")
                    nc.vector.reciprocal(rcp[:], rsum[:])
                    stash[("rcp", it)] = rcp

            def open_iter(it):
                b, h = iters[it]
                oT = wpool.tile([E, N], bf16, tag="oT", name=f"oT{it}")
                stash[("oT", it)] = oT
                if _is_t(it):
                    PTt = tptpool.tile([128, NT, N], bf16, tag="PTt",
                                       name=f"PTt{it}")
                    stash[("PTt", it)] = PTt
                else:
                    P = ppool.tile([128, 2, 4 * N], bf16, tag="P",
                                   name=f"P{it}")
                    rsum = wpool.tile([128, NT], f32, tag="rsum",
                                      name=f"rsum{it}")
                    stash[("Pr", it)] = (P, rsum)

            make_lhs(0)
            emitted_lhs = {0}
            with tc.high_priority(offset=-90):
                # all batch-0 v casts on DVE: the ScalarE variant's COPY ops
                # get scheduled between iteration 0's and 1's exps, blocking
                # the Scalar queue ~6us at that boundary.  (The DVE-queue
                # congestion they once dodged is gone now that make_lhs's
                # casts are emitted allocation-first at iteration tops.)
                v_proj(0, list(range(NT)))

            # batch-1 LN stats and tile work shifted to iterations 1..7 so
            # their DVE ops (which wait on the batch-1 x DMA) don't clog the
            # in-order DVE queue during iterations 0-1.  xnT[1] is needed by
            # make_lhs(8) (emitted at it=7), vall[1] by pv(8) at it=9.
            for it in range(NIT + 2):
                cur = it if it < NIT else None
                prev = it - 1 if 0 <= it - 1 < NIT else None
                # make_lhs(cur+1) FIRST: its PSUM tiles claim the earliest-
                # freed "m512" pool buffers and its PSUM->SBUF casts lead the
                # in-order DVE queue, so the pre-emitted boundary dots tile
                # of cur+1 has a ready lhs instead of stalling the PE ~2.5us
                # (which also drops it HAM-cold to half clock).
                if cur is not None and cur + 1 < NIT:
                    make_lhs(cur + 1)
                if it == 1:
                    ln_stats(1)
                # batch-1 LN tiles finish by iteration 6 so iteration 7's
                # DVE queue is clean ahead of make_lhs(8)'s casts (they fed
                # the 5us batch-switch stall when emitted at iter-7 top)
                if 1 <= it <= 5:
                    ln_apply(1, it - 1)
                    v_proj(1, [it - 1])
                if it == 6:
                    for t in (5, 6, 7):
                        ln_apply(1, t)
                        v_proj(1, [t])
                # the full rowsum tree for prev runs here: its inputs (the
                # previous iteration's exps) are long done, so it never
                # head-of-line-blocks the in-order DVE queue (emitting it
                # after the current halves, on fresh exps, cost +32us;
                # splitting L1/L23 across two windows measured no better)
                if prev is not None and _is_t(prev):
                    tree_l1(prev)
                    tree_l23(prev)
                if cur is not None:
                    pre = stash.pop(("pre", cur), 0)
                    if not pre:
                        open_iter(cur)
                    dots_half(cur, 0, prev, skip_n=pre)
                # mid-iteration: rowsum mini-matmuls of prev and the proj
                # block of it-2 land while the PE is ahead of ScalarE, not
                # in the iteration boundary where they'd delay the next
                # iteration's dots (and so the next exp).
                if prev is not None:
                    emit_rowsum(prev)
                if 0 <= it - 2 < NIT and it != NIT:
                    with tc.high_priority(offset=-60):
                        for t in range(NT):
                            proj_mm(it - 2, t)
                        for t in range(NT):
                            proj_ep(it - 2, t)
                        proj_drop(it - 2)
                if cur is not None:
                    dots_half(cur, 1, prev)
                # boundary pipelining: pre-emit the next iteration's first
                # dots tile so the PE has warm, ready work crossing the
                # iteration boundary and the next exp starts immediately.
                # (Emitting it earlier, inside the half, head-of-line-blocks
                # ScalarE: the new exp precedes this iteration's last exp in
                # the in-order queue and stalls it on a not-yet-ready dps.)
                if cur is not None and cur + 1 < NIT:
                    open_iter(cur + 1)
                    dots_tile(cur + 1, 0, [])
                    stash[("pre", cur + 1)] = 1
                if cur is None and prev is not None:
                    # tail drain: PV of the last iteration interleaved with
                    # the proj matmuls of it-2 (non-accumulating mms between
                    # accumulating PV mms dodge the PSUM RMW bubble)
                    pp = it - 2
                    pv_start(prev, 0)
                    pv_start(prev, 1)
                    for j in range(NT):
                        pv_mm(prev, 0, j)
                        proj_mm(pp, j)
                        pv_mm(prev, 1, j)
                        if j >= 1:
                            proj_ep(pp, j - 1)
                    pv_finish(prev, 0)
                    pv_finish(prev, 1)
                    proj_ep(pp, NT - 1)
                    proj_drop(pp)

    nc.compile()
    return nc


def _get_nc():
    if "nc" not in _cache:
        _cache["nc"] = _build_nc()
    return _cache["nc"]


def _make_in_maps(inputs):
    x = np.ascontiguousarray(np.asarray(inputs["x"], dtype=np.float32))
    ln_w = np.asarray(inputs["ln_w"], dtype=np.float32)
    ln_b = np.asarray(inputs["ln_b"], dtype=np.float32)
    w_qkv = np.asarray(inputs["w_qkv"], dtype=np.float32)
    scale = np.asarray(inputs["scale"], dtype=np.float32)
    w_proj = np.asarray(inputs["w_proj"], dtype=np.float32)

    INNER = E * H
    Wq = w_qkv[:, :INNER]
    Wk = w_qkv[:, INNER:2 * INNER]
    Wv = w_qkv[:, 2 * INNER:]

    amat_f = np.stack(
        [scale[h] * (Wq[:, h * E:(h + 1) * E] @ Wk[:, h * E:(h + 1) * E].T)
         for h in range(H)]
    )  # [H, E, E]
    amat = amat_f.astype(ml_dtypes.bfloat16)
    amatT = np.ascontiguousarray(
        amat_f.transpose(0, 2, 1)).astype(ml_dtypes.bfloat16)
    wvf = Wv.astype(ml_dtypes.bfloat16)  # [E, INNER]
    wp = w_proj.reshape(H, E, E).astype(ml_dtypes.bfloat16)  # [H, d, E]
    ident = np.eye(E, dtype=np.float32)
    identb = np.eye(E, dtype=ml_dtypes.bfloat16)
    negib = (np.eye(E, dtype=np.float32) * MASK_VAL).astype(
        ml_dtypes.bfloat16)
    lnw = ln_w.reshape(E, 1)
    lnb = ln_b.reshape(E, 1)

    b_proj = np.asarray(inputs["b_proj"], dtype=np.float32)
    bptile = np.broadcast_to(b_proj[None, :], (128, E)).copy()
    shared = {
        "amat": amat, "amatT": amatT, "wvf": wvf, "wp": wp,
        "bptile": bptile, "lnw": lnw, "lnb": lnb, "ident": ident,
        "identb": identb, "negib": negib,
    }
    return [
        {"x": x[c * B_LOC:(c + 1) * B_LOC], **shared} for c in range(NCORES)
    ]


def kernel(x, ln_w, ln_b, w_qkv, scale, w_proj, b_proj):
    from concourse.bass_utils import run_bass_kernel_spmd

    in_maps = _make_in_maps(dict(
        x=x, ln_w=ln_w, ln_b=ln_b, w_qkv=w_qkv, scale=scale,
        w_proj=w_proj, b_proj=b_proj,
    ))

    nc = _get_nc()
    res = run_bass_kernel_spmd(nc, in_maps, core_ids=list(range(NCORES)))
    y = np.concatenate([res.results[c]["out"] for c in range(NCORES)], axis=0)
    return y.astype(np.float32)
